# revision 35
# baseline (speedup 1.0000x reference)
"""CAM-module kernel for Trainium2, data-parallel over batch on 8 NeuronCores.

Per core (one batch sample, q = x[b] viewed as (C=512, N=4096) fp32):
  energy   = q @ q^T                      (C, C)   fp8 DoubleRow matmul, fp32 accum
  att[c,d] = softmax(max_d(energy) - energy)[c,d]
           = exp(m_c - e[c,d]) / Z_c      with m_c = row min of energy
  out      = gamma * (att @ q) + x

The row-max shift of the reference softmax cancels algebraically; only the
row minimum is needed for numerical stability (arguments of exp stay <= 0).
The gamma/Z row scale is folded into the exp bias:
  gamma/Z * exp(m - e) = exp(m + ln(gamma/Z) - e)
(ln(0) = -inf gives exact zeros when gamma == 0), so the attention@q result
only needs a single tensor add of x on its way out.
"""

import sys
import types

import numpy as np

import concourse.bass as bass
import concourse.tile as tile
from concourse import mybir
from concourse.masks import make_identity
from concourse.vector_clock import ScopedClock

P = 128
C = 512
N = 4096
B = 8
CT = C // P   # 4 c-tiles
KT = N // P   # 32 n-chunks of 128

STRIP_TAIL = False

FP32 = mybir.dt.float32
FP8 = mybir.dt.float8e4
DR = mybir.MatmulPerfMode.DoubleRow


def _drain_and_barrier_split(self, tick_clock, wait_clock):
    # The pinned walrus rejects >1 sync-wait on TPB_CTRL (Drain); spread the
    # final global-clock waits across a chain of drains, one wait each.
    nc = self.nc
    drain_inst = nc.sync.drain()
    wait_clock.add_sem_waits(
        drain_inst.ins, ScopedClock({None: tick_clock.global_clock})
    )
    si = drain_inst.ins.sync_info
    if si is not None and si.on_wait is not None and len(si.on_wait) > 1:
        waits = list(si.on_wait)
        si.on_wait = waits[:1]
        for w in waits[1:]:
            extra = nc.sync.drain()
            extra.ins.sync_info = mybir.SyncInfo(on_wait=[w], on_update=[])
    nc.all_engine_barrier()
    assert self.sems is not None
    popped = nc._tile_sem_poison_stack.pop()
    assert popped is self._sem_poison
    if not STRIP_TAIL:
        nc.clear_and_free_semaphores(list(self.sems.allocated().values()))
        nc.all_engine_barrier()


tile.TileContext._drain_and_barrier = _drain_and_barrier_split


def _legalize_sync_waits(nc):
    # This walrus build rejects instructions carrying more than one sync-wait.
    # Hoist extra waits onto same-engine NoOps placed immediately before the
    # instruction (engine streams preserve relative order within a block).
    for f in nc.m.functions:
        for bb in f.blocks:
            new = []
            for inst in bb.instructions:
                si = inst.sync_info
                if si is not None and si.on_wait and len(si.on_wait) > 1:
                    waits = list(si.on_wait)
                    for w in waits[:-1]:
                        nop = mybir.InstNoOp(
                            name=nc.get_next_instruction_name(),
                            engine=inst.engine,
                            bass_nofuse=True,
                            sync_info=mybir.SyncInfo(on_wait=[w], on_update=[]),
                        )
                        new.append(nop)
                    si.on_wait = [waits[-1]]
                new.append(inst)
            bb.instructions[:] = new


def build_nc():
    nc = bass.Bass()
    x_d = nc.declare_dram_parameter("x", [C, N], FP32, isOutput=False)
    g_d = nc.declare_dram_parameter("gamma", [1, 1], FP32, isOutput=False)
    o_d = nc.declare_dram_parameter("out", [C, N], FP32, isOutput=True)

    with tile.TileContext(nc) as tc:
        with (
            tc.tile_pool(name="singles", bufs=1) as singles,
            tc.tile_pool(name="stage", bufs=4) as stage,
            tc.tile_pool(name="psum_acc", bufs=4, space="PSUM") as psum_acc,
            tc.tile_pool(name="psum_tr", bufs=4, space="PSUM") as psum_tr,
        ):
            gcol = singles.tile([P, 1], FP32, tag="gamma")
            nc.gpsimd.dma_start(out=gcol[:], in_=g_d[:, :].to_broadcast((P, 1)))

            id8 = singles.tile([P, P], FP8, tag="id8")
            make_identity(nc, id8)
            lngam = singles.tile([P, 1], FP32, tag="lngam")
            nc.scalar.activation(
                out=lngam[:], in_=gcol[:], func=mybir.ActivationFunctionType.Ln
            )

            warm = singles.tile([P, P], FP32, tag="warm")
            for _ in range(32):
                wp = psum_tr.tile([P, P], FP32, tag="tr")
                nc.tensor.matmul(wp[:], lhsT=id8[:], rhs=id8[:], start=True, stop=True)
            nc.vector.tensor_copy(out=warm[:], in_=wp[:])

            xf = [
                singles.tile([P, N], FP32, tag=f"xf{ci}", name=f"xf{ci}")
                for ci in range(CT)
            ]
            q8 = singles.tile([P, CT, N], FP8, tag="q8")
            qT = singles.tile([P, KT, 512], FP8, tag="qT")
            e_ps = [
                psum_acc.tile([P, 512], FP32, tag="acc", name=f"e{ci}")
                for ci in range(CT)
            ]

            # Phase A/B interleaved: stream x in 1024-col super-groups, cast
            # to fp8 (ACT), transpose 128x128 blocks into qT (PE -> PSUM ->
            # DVE copy), accumulate DoubleRow energy matmuls (pairs of
            # 128-chunks -> K=256 per instruction).
            for gg in range(4):
                base = gg * 1024
                for ci in range(CT):
                    rows = slice(ci * P, (ci + 1) * P)
                    if gg == 0:
                        # finer first loads so the PE pipeline lights up sooner
                        for c0, c1 in ((0, 256), (256, 1024)):
                            nc.sync.dma_start(
                                out=xf[ci][:, base + c0 : base + c1],
                                in_=x_d[rows, base + c0 : base + c1],
                            )
                        nc.scalar.copy(
                            out=q8[:, ci, base : base + 256],
                            in_=xf[ci][:, base : base + 256],
                        )
                        nc.scalar.copy(
                            out=q8[:, ci, base + 256 : base + 512],
                            in_=xf[ci][:, base + 256 : base + 512],
                        )
                    else:
                        if gg == 1:
                            nc.sync.dma_start(
                                out=xf[ci][:, base : base + 2048],
                                in_=x_d[rows, base : base + 2048],
                            )
                        elif gg == 3:
                            nc.sync.dma_start(
                                out=xf[ci][:, base : base + 1024],
                                in_=x_d[rows, base : base + 1024],
                            )
                        nc.scalar.copy(
                            out=q8[:, ci, base : base + 512],
                            in_=xf[ci][:, base : base + 512],
                        )
                    nc.scalar.copy(
                        out=q8[:, ci, base + 512 : base + 1024],
                        in_=xf[ci][:, base + 512 : base + 1024],
                    )
                for tt in range(4):  # pairs of 128-chunks within super-group
                    t = gg * 4 + tt
                    for k in (2 * t, 2 * t + 1):
                        pt = psum_tr.tile([P, 512, 2], FP8, tag="tr")
                        for ci in range(CT):
                            nc.tensor.transpose(
                                pt[:, ci * P : (ci + 1) * P, 0],
                                q8[:, ci, k * P : (k + 1) * P],
                                id8[:],
                            )
                        nc.vector.tensor_copy(out=qT[:, k, :], in_=pt[:, :, 0])
                    for ci in range(CT):
                        nc.tensor.matmul(
                            e_ps[ci][:, ci * P :],
                            lhsT=qT[:, 2 * t : 2 * t + 2, ci * P : (ci + 1) * P],
                            rhs=qT[:, 2 * t : 2 * t + 2, ci * P :],
                            start=(t == 0),
                            stop=(t == KT // 2 - 1),
                            perf_mode=DR,
                        )

            # Mirror lower-triangle energy blocks from the symmetric upper
            # ones: e[ci][:, dj*128:] = transpose(e[dj][:, ci*128:]) for dj<ci.
            id32 = singles.tile([P, P], FP32, tag="id32")
            make_identity(nc, id32)
            for ci in range(1, CT):
                for dj in range(ci):
                    low = stage.tile([P, P], FP32, tag="low")
                    nc.vector.tensor_copy(
                        out=low[:], in_=e_ps[dj][:, ci * P : (ci + 1) * P]
                    )
                    nc.tensor.transpose(
                        e_ps[ci][:, dj * P : (dj + 1) * P], low[:], id32[:]
                    )

            # Per-ci: softmax (scale folded into exp bias), transpose the
            # scaled EXP row into EXPT columns, att@q, +x, and stream out.
            mcol = singles.tile([P, CT], FP32, tag="m")
            zcol = singles.tile([P, CT], FP32, tag="z")
            lnz = singles.tile([P, CT], FP32, tag="lnz")
            bias2 = singles.tile([P, CT], FP32, tag="bias2")
            EXPQ = singles.tile([P, CT, 512], FP8, tag="EXPQ")
            EXPT = singles.tile([P, CT, 512], FP8, tag="EXPT")

            # Emit all four softmax chains up front so the per-engine stages
            # pipeline across ci instead of serializing chain-by-chain.
            for ci in range(CT):
                cs = slice(ci, ci + 1)
                nc.vector.tensor_reduce(
                    out=mcol[:, cs],
                    in_=e_ps[ci][:],
                    axis=mybir.AxisListType.X,
                    op=mybir.AluOpType.min,
                )
                scr = stage.tile([P, 512], FP32, tag="scr")
                nc.scalar.activation(
                    out=scr[:],
                    in_=e_ps[ci][:],
                    func=mybir.ActivationFunctionType.Exp,
                    bias=mcol[:, cs],
                    scale=-1.0,
                    accum_out=zcol[:, cs],
                )
            nc.scalar.activation(
                out=lnz[:], in_=zcol[:], func=mybir.ActivationFunctionType.Ln
            )
            nc.vector.tensor_tensor(
                out=bias2[:], in0=mcol[:], in1=lnz[:], op=mybir.AluOpType.subtract
            )
            nc.vector.tensor_scalar(
                out=bias2[:],
                in0=bias2[:],
                scalar1=lngam[:],
                scalar2=None,
                op0=mybir.AluOpType.add,
            )
            for ci in range(CT):
                cs = slice(ci, ci + 1)
                nc.scalar.activation(
                    out=EXPQ[:, ci, :],
                    in_=e_ps[ci][:],
                    func=mybir.ActivationFunctionType.Exp,
                    bias=bias2[:, cs],
                    scale=-1.0,
                )
                for dj in range(CT):
                    ptx = psum_tr.tile([P, P, 2], FP8, tag="tr")
                    nc.tensor.transpose(
                        ptx[:, :, 0],
                        EXPQ[:, ci, dj * P : (dj + 1) * P],
                        id8[:],
                    )
                    nc.scalar.copy(
                        out=EXPT[:, dj, ci * P : (ci + 1) * P], in_=ptx[:, :, 0]
                    )

            for ci in range(CT):
                for nh in range(2):
                    osb = stage.tile([P, 2048], FP32, tag="osb")
                    for sub in range(4):
                        nj = nh * 4 + sub
                        po = psum_acc.tile([P, 512], FP32, tag="acc", name="po")
                        for j in range(2):
                            nc.tensor.matmul(
                                po[:],
                                lhsT=EXPT[:, 2 * j : 2 * j + 2, ci * P : (ci + 1) * P],
                                rhs=q8[:, 2 * j : 2 * j + 2, nj * 512 : (nj + 1) * 512],
                                start=(j == 0),
                                stop=(j == 1),
                                perf_mode=DR,
                            )
                        nc.vector.tensor_add(
                            out=osb[:, sub * 512 : (sub + 1) * 512],
                            in0=po[:],
                            in1=xf[ci][:, nj * 512 : (nj + 1) * 512],
                        )
                    if ci == CT - 1 and nh == 1:
                        nc.sync.dma_start(
                            out=o_d[ci * P : (ci + 1) * P, 2048:3072],
                            in_=osb[:, 0:1024],
                        )
                        nc.sync.dma_start(
                            out=o_d[ci * P : (ci + 1) * P, 3072:4096],
                            in_=osb[:, 1024:2048],
                        )
                    else:
                        nc.sync.dma_start(
                            out=o_d[ci * P : (ci + 1) * P, nh * 2048 : (nh + 1) * 2048],
                            in_=osb[:],
                        )
    _legalize_sync_waits(nc)
    return nc


def make_in_maps(x, gamma):
    x = np.ascontiguousarray(np.asarray(x, dtype=np.float32)).reshape(B, C, N)
    g = np.ascontiguousarray(np.asarray(gamma, dtype=np.float32)).reshape(1, 1)
    return [{"x": x[i], "gamma": g} for i in range(B)]


def kernel(x, y=None, gamma=None, **_ignored):
    from concourse.bass_utils import run_bass_kernel_spmd

    nc = build_nc()
    in_maps = make_in_maps(x, gamma)
    res = run_bass_kernel_spmd(nc, in_maps, list(range(B)))
    out = np.stack([np.asarray(res.results[i]["out"]) for i in range(B)])
    return out.reshape(B, C, 64, 64).astype(np.float32)


# revision 38
# speedup vs baseline: 1.0283x; 1.0283x over previous
"""CAM-module kernel for Trainium2, data-parallel over batch on 8 NeuronCores.

Per core (one batch sample, q = x[b] viewed as (C=512, N=4096) fp32):
  energy   = q @ q^T                      (C, C)   fp8 DoubleRow matmul, fp32 accum
  att[c,d] = softmax(max_d(energy) - energy)[c,d]
           = exp(m_c - e[c,d]) / Z_c      with m_c = row min of energy
  out      = gamma * (att @ q) + x

The row-max shift of the reference softmax cancels algebraically; only the
row minimum is needed for numerical stability (arguments of exp stay <= 0).
The gamma/Z row scale is folded into the exp bias:
  gamma/Z * exp(m - e) = exp(m + ln(gamma/Z) - e)
(ln(0) = -inf gives exact zeros when gamma == 0), so the attention@q result
only needs a single tensor add of x on its way out.
"""

import sys
import types

import numpy as np

import concourse.bass as bass
import concourse.tile as tile
from concourse import mybir
from concourse.masks import make_identity
from concourse.vector_clock import ScopedClock

P = 128
C = 512
N = 4096
B = 8
CT = C // P   # 4 c-tiles
KT = N // P   # 32 n-chunks of 128

STRIP_TAIL = False

FP32 = mybir.dt.float32
FP8 = mybir.dt.float8e4
DR = mybir.MatmulPerfMode.DoubleRow


def _drain_and_barrier_split(self, tick_clock, wait_clock):
    # The pinned walrus rejects >1 sync-wait on TPB_CTRL (Drain); spread the
    # final global-clock waits across a chain of drains, one wait each.
    nc = self.nc
    drain_inst = nc.sync.drain()
    wait_clock.add_sem_waits(
        drain_inst.ins, ScopedClock({None: tick_clock.global_clock})
    )
    si = drain_inst.ins.sync_info
    if si is not None and si.on_wait is not None and len(si.on_wait) > 1:
        waits = list(si.on_wait)
        si.on_wait = waits[:1]
        for w in waits[1:]:
            extra = nc.sync.drain()
            extra.ins.sync_info = mybir.SyncInfo(on_wait=[w], on_update=[])
    nc.all_engine_barrier()
    assert self.sems is not None
    popped = nc._tile_sem_poison_stack.pop()
    assert popped is self._sem_poison
    if not STRIP_TAIL:
        nc.clear_and_free_semaphores(list(self.sems.allocated().values()))
        nc.all_engine_barrier()


tile.TileContext._drain_and_barrier = _drain_and_barrier_split


def _legalize_sync_waits(nc):
    # This walrus build rejects instructions carrying more than one sync-wait.
    # Hoist extra waits onto same-engine NoOps placed immediately before the
    # instruction (engine streams preserve relative order within a block).
    for f in nc.m.functions:
        for bb in f.blocks:
            new = []
            for inst in bb.instructions:
                si = inst.sync_info
                if si is not None and si.on_wait and len(si.on_wait) > 1:
                    waits = list(si.on_wait)
                    for w in waits[:-1]:
                        nop = mybir.InstNoOp(
                            name=nc.get_next_instruction_name(),
                            engine=inst.engine,
                            bass_nofuse=True,
                            sync_info=mybir.SyncInfo(on_wait=[w], on_update=[]),
                        )
                        new.append(nop)
                    si.on_wait = [waits[-1]]
                new.append(inst)
            bb.instructions[:] = new


def build_nc():
    nc = bass.Bass()
    x_d = nc.declare_dram_parameter("x", [C, N], FP32, isOutput=False)
    g_d = nc.declare_dram_parameter("gamma", [1, 1], FP32, isOutput=False)
    o_d = nc.declare_dram_parameter("out", [C, N], FP32, isOutput=True)

    with tile.TileContext(nc) as tc:
        with (
            tc.tile_pool(name="singles", bufs=1) as singles,
            tc.tile_pool(name="stage", bufs=4) as stage,
            tc.tile_pool(name="psum_acc", bufs=4, space="PSUM") as psum_acc,
            tc.tile_pool(name="psum_tr", bufs=4, space="PSUM") as psum_tr,
        ):
            gcol = singles.tile([P, 1], FP32, tag="gamma")
            nc.gpsimd.dma_start(out=gcol[:], in_=g_d[:, :].to_broadcast((P, 1)))

            id8 = singles.tile([P, P], FP8, tag="id8")
            make_identity(nc, id8)
            lngam = singles.tile([P, 1], FP32, tag="lngam")
            nc.scalar.activation(
                out=lngam[:], in_=gcol[:], func=mybir.ActivationFunctionType.Ln
            )

            warm = singles.tile([P, P], FP32, tag="warm")
            for _ in range(32):
                wp = psum_tr.tile([P, P], FP32, tag="tr")
                nc.tensor.matmul(wp[:], lhsT=id8[:], rhs=id8[:], start=True, stop=True)
            nc.vector.tensor_copy(out=warm[:], in_=wp[:])

            xf = [
                singles.tile([P, N], FP32, tag=f"xf{ci}", name=f"xf{ci}")
                for ci in range(CT)
            ]
            q8 = singles.tile([P, CT, N], FP8, tag="q8")
            qT = singles.tile([P, KT, 512], FP8, tag="qT")
            e_ps = [
                psum_acc.tile([P, 512], FP32, tag="acc", name=f"e{ci}")
                for ci in range(CT)
            ]

            # Phase A/B interleaved: stream x in 1024-col super-groups, cast
            # to fp8 (ACT), transpose 128x128 blocks into qT (PE -> PSUM ->
            # DVE copy), accumulate DoubleRow energy matmuls (pairs of
            # 128-chunks -> K=256 per instruction).
            for gg in range(4):
                base = gg * 1024
                for ci in range(CT):
                    rows = slice(ci * P, (ci + 1) * P)
                    if gg == 0:
                        # finer first loads so the PE pipeline lights up sooner
                        for c0, c1 in ((0, 256), (256, 1024)):
                            nc.sync.dma_start(
                                out=xf[ci][:, base + c0 : base + c1],
                                in_=x_d[rows, base + c0 : base + c1],
                            )
                        nc.scalar.copy(
                            out=q8[:, ci, base : base + 256],
                            in_=xf[ci][:, base : base + 256],
                        )
                        nc.scalar.copy(
                            out=q8[:, ci, base + 256 : base + 512],
                            in_=xf[ci][:, base + 256 : base + 512],
                        )
                    else:
                        nc.sync.dma_start(
                            out=xf[ci][:, base : base + 1024],
                            in_=x_d[rows, base : base + 1024],
                        )
                        nc.scalar.copy(
                            out=q8[:, ci, base : base + 512],
                            in_=xf[ci][:, base : base + 512],
                        )
                    nc.scalar.copy(
                        out=q8[:, ci, base + 512 : base + 1024],
                        in_=xf[ci][:, base + 512 : base + 1024],
                    )
                for tt in range(4):  # pairs of 128-chunks within super-group
                    t = gg * 4 + tt
                    for k in (2 * t, 2 * t + 1):
                        pt = psum_tr.tile([P, 512, 2], FP8, tag="tr")
                        for ci in range(CT):
                            nc.tensor.transpose(
                                pt[:, ci * P : (ci + 1) * P, 0],
                                q8[:, ci, k * P : (k + 1) * P],
                                id8[:],
                            )
                        nc.vector.tensor_copy(out=qT[:, k, :], in_=pt[:, :, 0])
                    for ci in range(CT):
                        nc.tensor.matmul(
                            e_ps[ci][:, ci * P :],
                            lhsT=qT[:, 2 * t : 2 * t + 2, ci * P : (ci + 1) * P],
                            rhs=qT[:, 2 * t : 2 * t + 2, ci * P :],
                            start=(t == 0),
                            stop=(t == KT // 2 - 1),
                            perf_mode=DR,
                        )

            # Mirror lower-triangle energy blocks from the symmetric upper
            # ones: e[ci][:, dj*128:] = transpose(e[dj][:, ci*128:]) for dj<ci.
            id32 = singles.tile([P, P], FP32, tag="id32")
            make_identity(nc, id32)
            for ci in range(1, CT):
                for dj in range(ci):
                    low = stage.tile([P, P], FP32, tag="low")
                    nc.vector.tensor_copy(
                        out=low[:], in_=e_ps[dj][:, ci * P : (ci + 1) * P]
                    )
                    nc.tensor.transpose(
                        e_ps[ci][:, dj * P : (dj + 1) * P], low[:], id32[:]
                    )

            # Per-ci: softmax (scale folded into exp bias), transpose the
            # scaled EXP row into EXPT columns, att@q, +x, and stream out.
            mcol = singles.tile([P, CT], FP32, tag="m")
            zcol = singles.tile([P, CT], FP32, tag="z")
            lnz = singles.tile([P, CT], FP32, tag="lnz")
            bias2 = singles.tile([P, CT], FP32, tag="bias2")
            EXPQ = singles.tile([P, CT, 512], FP8, tag="EXPQ")
            EXPT = singles.tile([P, CT, 512], FP8, tag="EXPT")

            # Emit all four softmax chains up front so the per-engine stages
            # pipeline across ci instead of serializing chain-by-chain.
            for ci in range(CT):
                cs = slice(ci, ci + 1)
                nc.vector.tensor_reduce(
                    out=mcol[:, cs],
                    in_=e_ps[ci][:],
                    axis=mybir.AxisListType.X,
                    op=mybir.AluOpType.min,
                )
                scr = stage.tile([P, 512], FP32, tag="scr")
                nc.scalar.activation(
                    out=scr[:],
                    in_=e_ps[ci][:],
                    func=mybir.ActivationFunctionType.Exp,
                    bias=mcol[:, cs],
                    scale=-1.0,
                    accum_out=zcol[:, cs],
                )
            for ci in range(CT):
                cs = slice(ci, ci + 1)
                nc.scalar.activation(
                    out=lnz[:, cs],
                    in_=zcol[:, cs],
                    func=mybir.ActivationFunctionType.Ln,
                )
                nc.vector.tensor_scalar(
                    out=bias2[:, cs],
                    in0=mcol[:, cs],
                    scalar1=lnz[:, cs],
                    scalar2=lngam[:],
                    op0=mybir.AluOpType.subtract,
                    op1=mybir.AluOpType.add,
                )
            for ci in range(CT):
                cs = slice(ci, ci + 1)
                nc.scalar.activation(
                    out=EXPQ[:, ci, :],
                    in_=e_ps[ci][:],
                    func=mybir.ActivationFunctionType.Exp,
                    bias=bias2[:, cs],
                    scale=-1.0,
                )
                for dj in range(CT):
                    ptx = psum_tr.tile([P, P, 2], FP8, tag="tr")
                    nc.tensor.transpose(
                        ptx[:, :, 0],
                        EXPQ[:, ci, dj * P : (dj + 1) * P],
                        id8[:],
                    )
                    nc.scalar.copy(
                        out=EXPT[:, dj, ci * P : (ci + 1) * P], in_=ptx[:, :, 0]
                    )

            for ci in range(CT):
                for nh in range(2):
                    osb = stage.tile([P, 2048], FP32, tag="osb")
                    for sub in range(4):
                        nj = nh * 4 + sub
                        po = psum_acc.tile([P, 512], FP32, tag="acc", name="po")
                        for j in range(2):
                            nc.tensor.matmul(
                                po[:],
                                lhsT=EXPT[:, 2 * j : 2 * j + 2, ci * P : (ci + 1) * P],
                                rhs=q8[:, 2 * j : 2 * j + 2, nj * 512 : (nj + 1) * 512],
                                start=(j == 0),
                                stop=(j == 1),
                                perf_mode=DR,
                            )
                        nc.vector.tensor_add(
                            out=osb[:, sub * 512 : (sub + 1) * 512],
                            in0=po[:],
                            in1=xf[ci][:, nj * 512 : (nj + 1) * 512],
                        )
                    if ci == CT - 1 and nh == 1:
                        nc.sync.dma_start(
                            out=o_d[ci * P : (ci + 1) * P, 2048:3072],
                            in_=osb[:, 0:1024],
                        )
                        nc.sync.dma_start(
                            out=o_d[ci * P : (ci + 1) * P, 3072:4096],
                            in_=osb[:, 1024:2048],
                        )
                    else:
                        nc.sync.dma_start(
                            out=o_d[ci * P : (ci + 1) * P, nh * 2048 : (nh + 1) * 2048],
                            in_=osb[:],
                        )
    _legalize_sync_waits(nc)
    return nc


def make_in_maps(x, gamma):
    x = np.ascontiguousarray(np.asarray(x, dtype=np.float32)).reshape(B, C, N)
    g = np.ascontiguousarray(np.asarray(gamma, dtype=np.float32)).reshape(1, 1)
    return [{"x": x[i], "gamma": g} for i in range(B)]


def kernel(x, y=None, gamma=None, **_ignored):
    from concourse.bass_utils import run_bass_kernel_spmd

    nc = build_nc()
    in_maps = make_in_maps(x, gamma)
    res = run_bass_kernel_spmd(nc, in_maps, list(range(B)))
    out = np.stack([np.asarray(res.results[i]["out"]) for i in range(B)])
    return out.reshape(B, C, 64, 64).astype(np.float32)
